# revision 7
# baseline (speedup 1.0000x reference)
"""MoE grouped-GEMM (router + top-2 combine + per-expert FFN) on 8 TRN2 NeuronCores.

Expert parallelism with token gather ("all-to-all tokens by expert assignment"):
the router (linear -> softmax -> top-2) runs host-side as part of the shard
step; core c owns expert c (weights1[c], weights2[c]) and receives ONLY the
tokens routed to expert c, padded to a common capacity C (max expert load
rounded up to 32). Each core computes its expert's FFN for its gathered
tokens; the host applies the combine weights and scatter-adds the 8 partial
outputs back to token order (the unshard step).

This cuts device FLOPs 4x vs the dense-over-experts formulation: only
top-2-of-8 expert-token pairs are computed (2048*2 = 4096 pairs vs 2048*8).

Problem shapes (hardcoded): tokens [2048, 1024] f32, router_w [8, 1024],
weights1 [8, 1024, 1024], weights2 [8, 1024, 1024], out [2048, 1024].

Per-core device program (SPMD, differs only via inputs):
  tokG [128, 8, C]     gathered tokens bf16, pre-tiled host-side so the
                       contraction dim d lands on SBUF partitions (p = d%128,
                       a = d//128) and every DMA run is >= 1KB contiguous.
  w1   [8, 128, 1024]  weights1[c] pre-tiled as [j, p, (a hh)] so a per-j
                       chunk DMA is one contiguous 2KB run per partition.
  w2   [8, 128, 1024]  weights2[c] * 0.5, pre-tiled as [o, p, (j oo)].
  FFN: hT[j, t] = x * (1 + erf(x/sqrt(2))),  x = sum_d w1[d, j] tokG[d, t]
       yT[o, t] = sum_j hT[j, t] w2[j, o]
  out  [8, 128, C]     yT bf16 (combine weights applied host-side).

Both GEMMs keep tokens as the moving operand (512-max free dim), so a
non-multiple-of-128 capacity wastes nothing on the PE. Input DMAs are
prefetch-ordered on the SP HWDGE queue (w1 j-chunk 0 + first tokens first);
w2 streams concurrently on the Activation HWDGE queue.
"""

import os
import sys

import numpy as np

for _p in ("/opt/trn_rl_repo", "/root/.axon_site/_ro/trn_rl_repo"):
    if os.path.isdir(_p) and _p not in sys.path:
        sys.path.insert(0, _p)

from contextlib import ExitStack

import concourse.bass as bass
import concourse.tile as tile
from concourse import bacc, mybir
from concourse.bass_utils import run_bass_kernel_spmd

F32 = mybir.dt.float32
BF16 = mybir.dt.bfloat16
AF = mybir.ActivationFunctionType
ALU = mybir.AluOpType

T = 2048  # tokens
D = 1024  # input dim
H = 1024  # hidden dim
O = 1024  # output dim
E = 8  # experts == cores
P = 128  # partitions
KD = D // P  # 8 contraction tiles (d)
JT = H // P  # 8 contraction tiles (j)
OT = O // P  # 8 output tiles (o)
_NCORES = 8


def _blocks(C):
    """Split C tokens into balanced moving-dim blocks of <= 512 (PSUM bank)."""
    nb = -(-C // 512)
    tb = -(-(-(-C // nb)) // 32) * 32  # ceil(C/nb) rounded up to 32
    out = []
    s = 0
    while s < C:
        out.append((s, min(tb, C - s)))
        s += tb
    return out


def _emit(tc, aps, act_fn, C):
    nc = tc.nc
    tokd = aps["tokG"]  # [P, KD, C]
    w1d = aps["w1"].rearrange("j p x -> p j x")  # [P, JT, KD*128]
    w2d = aps["w2"].rearrange("o p x -> p o x")  # [P, OT, JT*128]
    outd = aps["out"].rearrange("o p t -> p o t")  # [P, OT, C]
    blocks = _blocks(C)

    with ExitStack() as ctx:
        wp = ctx.enter_context(tc.tile_pool(name="wp", bufs=1))
        hp = ctx.enter_context(tc.tile_pool(name="hp", bufs=1))
        yp = ctx.enter_context(tc.tile_pool(name="yp", bufs=6))
        ph = ctx.enter_context(tc.tile_pool(name="ph", bufs=4, space="PSUM"))
        py = ctx.enter_context(tc.tile_pool(name="py", bufs=4, space="PSUM"))

        tok_sb = wp.tile([P, KD, C], BF16)
        w1_sb = wp.tile([P, JT, KD * P], BF16)
        w2_sb = wp.tile([P, OT, JT * P], BF16)

        # PE warmup: the Tensor engine ramps its clock only after ~3us of
        # continuous execution. Dummy matmuls on a zeroed scratch tile keep
        # the PE busy during the initial DMA wait so the real GEMMs start at
        # (or near) full clock.
        wu_sb = wp.tile([P, 64], BF16)
        nc.vector.memset(wu_sb[:], 0.0)
        for _ in range(28):
            psum_wu = ph.tile([P, 64], F32, name="psum_h")
            nc.tensor.matmul(
                psum_wu[0:64, 0:64], lhsT=wu_sb[:, 0:64], rhs=wu_sb[:, 0:64],
                start=True, stop=True,
            )

        # Input DMAs. SP queue carries the PE-critical stream in consumption
        # order: w1 j-chunk 0, tokens a0-1, tokens a2-7, w1 j-chunks 1-7 (one
        # merged transfer; descriptors are generated in j order). Activation
        # queue concurrently carries w2 (first needed only at GEMM2) and the
        # output stores.
        nc.sync.dma_start(w1_sb[:, 0, :], w1d[:, 0, :])
        nc.sync.dma_start(tok_sb[:, 0:2, :], tokd[:, 0:2, :])
        nc.sync.dma_start(tok_sb[:, 2:KD, :], tokd[:, 2:KD, :])
        nc.sync.dma_start(w1_sb[:, 1:JT, :], w1d[:, 1:JT, :])
        nc.scalar.dma_start(w2_sb[:], w2d)

        # ---- GEMM1: hT[j, t] = act(sum_d w1[d, j] tokG[d, t]) ----
        # Exact gelu(x) = 0.5*x*(1 + erf(x/sqrt(2))); the 0.5 is folded into
        # w2 host-side, so on-device: h = x * (1 + erf(x/sqrt(2))).
        # Token blocks are interleaved inside the a-loop so consecutive
        # matmuls share the same stationary operand (cheaper weight reloads).
        h_sb = [
            hp.tile([P, JT, tb], BF16, name=f"h_sb{bi}")
            for bi, (_, tb) in enumerate(blocks)
        ]
        for j in range(JT):
            psum_h = [ph.tile([P, tb], F32, name="psum_h") for _, tb in blocks]
            for a in range(KD):
                for bi, (bs, tb) in enumerate(blocks):
                    nc.tensor.matmul(
                        psum_h[bi][:],
                        lhsT=w1_sb[:, j, a * P : (a + 1) * P],
                        rhs=tok_sb[:, a, bs : bs + tb],
                        start=(a == 0),
                        stop=(a == KD - 1),
                        skip_group_check=True,
                    )
            for bi, (bs, tb) in enumerate(blocks):
                e_sb = yp.tile([P, tb], F32, name="e_sb")
                nc.scalar.activation(
                    e_sb[:], psum_h[bi][:], act_fn, scale=0.7071067811865476
                )
                nc.vector.scalar_tensor_tensor(
                    h_sb[bi][:, j, :], e_sb[:], 1.0, psum_h[bi][:],
                    op0=ALU.add, op1=ALU.mult,
                )

        # ---- GEMM2: yT[o, t] = sum_j hT[j, t] w2[j, o] ----
        for o in range(OT):
            psum_y = [py.tile([P, tb], F32, name="psum_y") for _, tb in blocks]
            for j in range(JT):
                for bi, (bs, tb) in enumerate(blocks):
                    nc.tensor.matmul(
                        psum_y[bi][:],
                        lhsT=w2_sb[:, o, j * P : (j + 1) * P],
                        rhs=h_sb[bi][:, j, :],
                        start=(j == 0),
                        stop=(j == JT - 1),
                        skip_group_check=True,
                    )
            y_sb = yp.tile([P, C], BF16, name="y_sb")
            for bi, (bs, tb) in enumerate(blocks):
                nc.scalar.copy(y_sb[:, bs : bs + tb], psum_y[bi][:])
            nc.scalar.dma_start(outd[:, o, :], y_sb[:])


def build(C, sim_act=False):
    """Build + compile the SPMD program for token capacity C. sim_act=True
    swaps the FFN activation to Tanh so CoreSim (which lacks Erf) can run."""
    nc = bacc.Bacc(
        "TRN2", target_bir_lowering=False, debug=False, num_devices=_NCORES
    )
    aps = {
        "tokG": nc.dram_tensor("tokG", [P, KD, C], BF16, kind="ExternalInput").ap(),
        "w1": nc.dram_tensor("w1", [JT, P, KD * P], BF16, kind="ExternalInput").ap(),
        "w2": nc.dram_tensor("w2", [OT, P, JT * P], BF16, kind="ExternalInput").ap(),
        "out": nc.dram_tensor("out", [OT, P, C], BF16, kind="ExternalOutput").ap(),
    }
    act = AF.Tanh if sim_act else AF.Erf
    with tile.TileContext(nc) as tc:
        _emit(tc, aps, act, C)
    nc.compile()
    return nc


def _route(tokens, router_w):
    """Host router in float64: linear -> softmax -> top-2. Margins on this
    input are ~1e-4, far above f32 eps, so selection matches the f32 ref."""
    logits = tokens.astype(np.float64) @ router_w.astype(np.float64).T  # [T, E]
    e = np.exp(logits - logits.max(axis=1, keepdims=True))
    scores = e / e.sum(axis=1, keepdims=True)
    order = np.argsort(scores, axis=1)
    ind = np.zeros_like(scores)
    np.put_along_axis(ind, order[:, -2:], 1.0, axis=1)
    return scores * ind  # comb [T, E]


_NC_CACHE = {}


def kernel(tokens, router_w, weights1, weights2, trace=False):
    import ml_dtypes

    tokens = np.ascontiguousarray(np.asarray(tokens, dtype=np.float32))
    router_w = np.ascontiguousarray(np.asarray(router_w, dtype=np.float32))
    weights1 = np.asarray(weights1, dtype=np.float32)
    weights2 = np.asarray(weights2, dtype=np.float32)
    assert tokens.shape == (T, D) and router_w.shape == (E, D)
    assert weights1.shape == (E, D, H) and weights2.shape == (E, H, O)

    comb = _route(tokens, router_w)  # [T, E] float64
    idx = [np.nonzero(comb[:, c])[0] for c in range(E)]
    maxL = max(len(i) for i in idx)
    C = max(64, -(-maxL // 32) * 32)

    if C not in _NC_CACHE:
        _NC_CACHE[C] = build(C)
    nc = _NC_CACHE[C]

    bf16 = ml_dtypes.bfloat16
    in_maps = []
    for c in range(E):
        # tokens pre-tiled to [p, a, t]: tokG[p, a, s] = tokens[idx[s], a*128+p]
        tokG = np.zeros((P, KD, C), dtype=bf16)
        g = tokens[idx[c]].T.reshape(KD, P, len(idx[c])).transpose(1, 0, 2)
        tokG[:, :, : len(idx[c])] = g.astype(bf16)
        # w1 pre-tiled to [j, p, (a hh)]: w1t[j, p, a*128+hh] = w1[a*128+p, j*128+hh]
        w1t = (
            weights1[c]
            .reshape(KD, P, JT, P)
            .transpose(2, 1, 0, 3)
            .reshape(JT, P, KD * P)
        )
        # w2 pre-tiled to [o, p, (j oo)], with the gelu 0.5 folded in
        w2t = (
            (weights2[c] * 0.5)
            .reshape(JT, P, OT, P)
            .transpose(2, 1, 0, 3)
            .reshape(OT, P, JT * P)
        )
        in_maps.append(
            {
                "tokG": tokG,
                "w1": np.ascontiguousarray(w1t).astype(bf16),
                "w2": np.ascontiguousarray(w2t).astype(bf16),
            }
        )

    res = run_bass_kernel_spmd(nc, in_maps, list(range(_NCORES)), trace=trace)
    out = np.zeros((T, O), dtype=np.float64)
    for c in range(E):
        yT = np.asarray(res.results[c]["out"]).astype(np.float64)  # [OT, P, C]
        L = len(idx[c])
        y = yT.reshape(O, C)[:, :L]
        out[idx[c]] += comb[idx[c], c : c + 1] * y.T
    if trace:
        kernel.last_results = res
    return out.astype(np.float32)


# revision 8
# speedup vs baseline: 1.1090x; 1.1090x over previous
"""MoE grouped-GEMM (router + top-2 combine + per-expert FFN) on 8 TRN2 NeuronCores.

Expert parallelism with token gather ("all-to-all tokens by expert assignment"):
the router (linear -> softmax -> top-2) runs host-side as part of the shard
step; core c owns expert c (weights1[c], weights2[c]) and receives ONLY the
tokens routed to expert c, padded to a common capacity C (max expert load
rounded up to 32). Each core computes its expert's FFN for its gathered
tokens; the host applies the combine weights and scatter-adds the 8 partial
outputs back to token order (the unshard step).

This cuts device FLOPs 4x vs the dense-over-experts formulation: only
top-2-of-8 expert-token pairs are computed (2048*2 = 4096 pairs vs 2048*8).

Problem shapes (hardcoded): tokens [2048, 1024] f32, router_w [8, 1024],
weights1 [8, 1024, 1024], weights2 [8, 1024, 1024], out [2048, 1024].

Per-core device program (SPMD, differs only via inputs):
  tokG [128, 8, C]     gathered tokens bf16, pre-tiled host-side so the
                       contraction dim d lands on SBUF partitions (p = d%128,
                       a = d//128) and every DMA run is >= 1KB contiguous.
  w1   [8, 128, 1024]  weights1[c] pre-tiled as [j, p, (a hh)] so a per-j
                       chunk DMA is one contiguous 2KB run per partition.
  w2   [8, 128, 1024]  weights2[c] * 0.5, pre-tiled as [o, p, (j oo)].
  FFN: hT[j, t] = x * (1 + erf(x/sqrt(2))),  x = sum_d w1[d, j] tokG[d, t]
       yT[o, t] = sum_j hT[j, t] w2[j, o]
  out  [8, 128, C]     yT bf16 (combine weights applied host-side).

Both GEMMs keep tokens as the moving operand (512-max free dim), so a
non-multiple-of-128 capacity wastes nothing on the PE. Input DMAs are
prefetch-ordered on the SP HWDGE queue (w1 j-chunk 0 + first tokens first);
w2 streams concurrently on the Activation HWDGE queue.
"""

import os
import sys

import numpy as np

for _p in ("/opt/trn_rl_repo", "/root/.axon_site/_ro/trn_rl_repo"):
    if os.path.isdir(_p) and _p not in sys.path:
        sys.path.insert(0, _p)

from contextlib import ExitStack

import concourse.bass as bass
import concourse.tile as tile
from concourse import bacc, mybir
from concourse.bass_utils import run_bass_kernel_spmd

F32 = mybir.dt.float32
BF16 = mybir.dt.bfloat16
AF = mybir.ActivationFunctionType
ALU = mybir.AluOpType

T = 2048  # tokens
D = 1024  # input dim
H = 1024  # hidden dim
O = 1024  # output dim
E = 8  # experts == cores
P = 128  # partitions
KD = D // P  # 8 contraction tiles (d)
JT = H // P  # 8 contraction tiles (j)
OT = O // P  # 8 output tiles (o)
_NCORES = 8


def _blocks(C):
    """Split C tokens into moving-dim blocks of <= 512 (PSUM bank limit),
    biggest first: a large block 0 makes GEMM1's j-cycle slower than the w1
    j-chunk DMA feed, so the PE never stalls on weight arrival."""
    out = []
    s = 0
    while s < C:
        tb = min(512, C - s)
        out.append((s, tb))
        s += tb
    return out


def _emit(tc, aps, act_fn, C):
    nc = tc.nc
    tokd = aps["tokG"]  # [P, KD, C]
    w1d = aps["w1"].rearrange("j p x -> p j x")  # [P, JT, KD*128]
    w2d = aps["w2"].rearrange("o p x -> p o x")  # [P, OT, JT*128]
    outd = aps["out"].rearrange("o p t -> p o t")  # [P, OT, C]
    blocks = _blocks(C)

    with ExitStack() as ctx:
        wp = ctx.enter_context(tc.tile_pool(name="wp", bufs=1))
        hp = ctx.enter_context(tc.tile_pool(name="hp", bufs=1))
        yp = ctx.enter_context(tc.tile_pool(name="yp", bufs=6))
        ph = ctx.enter_context(tc.tile_pool(name="ph", bufs=4, space="PSUM"))
        py = ctx.enter_context(tc.tile_pool(name="py", bufs=4, space="PSUM"))

        tok_sb = wp.tile([P, KD, C], BF16)
        w1_sb = wp.tile([P, JT, KD * P], BF16)
        w2_sb = wp.tile([P, OT, JT * P], BF16)

        # PE warmup: the Tensor engine ramps its clock only after ~3us of
        # continuous execution. Dummy matmuls on a zeroed scratch tile keep
        # the PE busy during the initial DMA wait so the real GEMMs start at
        # (or near) full clock.
        wu_sb = wp.tile([P, 640], BF16)
        nc.vector.memset(wu_sb[:], 0.0)
        for _ in range(6):
            psum_wu = ph.tile([P, 512], F32, name="psum_h")
            nc.tensor.matmul(
                psum_wu[:], lhsT=wu_sb[:, 0:P], rhs=wu_sb[:, P : P + 512],
                start=True, stop=True,
            )

        # Input DMAs, split across the two HWDGE queues so both engine sets
        # pull concurrently, each in consumption order:
        #   SP queue:  w1 j-chunks 0..7 (GEMM1 stationaries), w2 o-chunks 4-7
        #   Act queue: block-0 tokens (a0-1 first so the first accumulation
        #              can start early), remaining tokens, w2 o-chunks 0-3,
        #              then the output stores emitted by the GEMM2 loop.
        for j in range(JT):
            nc.sync.dma_start(w1_sb[:, j, :], w1d[:, j, :])
        nc.sync.dma_start(w2_sb[:, 4:OT, :], w2d[:, 4:OT, :])
        b0, tb0 = blocks[0]
        nc.scalar.dma_start(tok_sb[:, 0:2, 0:tb0], tokd[:, 0:2, 0:tb0])
        nc.scalar.dma_start(tok_sb[:, 2:KD, 0:tb0], tokd[:, 2:KD, 0:tb0])
        for bs, tb in blocks[1:]:
            nc.scalar.dma_start(tok_sb[:, :, bs : bs + tb], tokd[:, :, bs : bs + tb])
        nc.scalar.dma_start(w2_sb[:, 0:4, :], w2d[:, 0:4, :])

        # ---- GEMM1: hT[j, t] = act(sum_d w1[d, j] tokG[d, t]) ----
        # Exact gelu(x) = 0.5*x*(1 + erf(x/sqrt(2))); the 0.5 is folded into
        # w2 host-side, so on-device: h = x * (1 + erf(x/sqrt(2))).
        h_sb = [
            hp.tile([P, JT, tb], BF16, name=f"h_sb{bi}")
            for bi, (_, tb) in enumerate(blocks)
        ]
        for bi, (bs, tb) in enumerate(blocks):
            for j in range(JT):
                psum_h = ph.tile([P, tb], F32, name="psum_h")
                for a in range(KD):
                    nc.tensor.matmul(
                        psum_h[:],
                        lhsT=w1_sb[:, j, a * P : (a + 1) * P],
                        rhs=tok_sb[:, a, bs : bs + tb],
                        start=(a == 0),
                        stop=(a == KD - 1),
                    )
                e_sb = yp.tile([P, tb], F32, name="e_sb")
                nc.scalar.activation(
                    e_sb[:], psum_h[:], act_fn, scale=0.7071067811865476
                )
                nc.vector.scalar_tensor_tensor(
                    h_sb[bi][:, j, :], e_sb[:], 1.0, psum_h[:],
                    op0=ALU.add, op1=ALU.mult,
                )

        # ---- GEMM2: yT[o, t] = sum_j hT[j, t] w2[j, o] ----
        for o in range(OT):
            y_sb = yp.tile([P, C], BF16, name="y_sb")
            for bi, (bs, tb) in enumerate(blocks):
                psum_y = py.tile([P, tb], F32, name="psum_y")
                for j in range(JT):
                    nc.tensor.matmul(
                        psum_y[:],
                        lhsT=w2_sb[:, o, j * P : (j + 1) * P],
                        rhs=h_sb[bi][:, j, :],
                        start=(j == 0),
                        stop=(j == JT - 1),
                    )
                nc.scalar.copy(y_sb[:, bs : bs + tb], psum_y[:])
            nc.scalar.dma_start(outd[:, o, :], y_sb[:])


def build(C, sim_act=False):
    """Build + compile the SPMD program for token capacity C. sim_act=True
    swaps the FFN activation to Tanh so CoreSim (which lacks Erf) can run."""
    nc = bacc.Bacc(
        "TRN2", target_bir_lowering=False, debug=False, num_devices=_NCORES
    )
    aps = {
        "tokG": nc.dram_tensor("tokG", [P, KD, C], BF16, kind="ExternalInput").ap(),
        "w1": nc.dram_tensor("w1", [JT, P, KD * P], BF16, kind="ExternalInput").ap(),
        "w2": nc.dram_tensor("w2", [OT, P, JT * P], BF16, kind="ExternalInput").ap(),
        "out": nc.dram_tensor("out", [OT, P, C], BF16, kind="ExternalOutput").ap(),
    }
    act = AF.Tanh if sim_act else AF.Erf
    with tile.TileContext(nc) as tc:
        _emit(tc, aps, act, C)
    nc.compile()
    return nc


def _route(tokens, router_w):
    """Host router in float64: linear -> softmax -> top-2. Margins on this
    input are ~1e-4, far above f32 eps, so selection matches the f32 ref."""
    logits = tokens.astype(np.float64) @ router_w.astype(np.float64).T  # [T, E]
    e = np.exp(logits - logits.max(axis=1, keepdims=True))
    scores = e / e.sum(axis=1, keepdims=True)
    order = np.argsort(scores, axis=1)
    ind = np.zeros_like(scores)
    np.put_along_axis(ind, order[:, -2:], 1.0, axis=1)
    return scores * ind  # comb [T, E]


_NC_CACHE = {}


def kernel(tokens, router_w, weights1, weights2, trace=False):
    import ml_dtypes

    tokens = np.ascontiguousarray(np.asarray(tokens, dtype=np.float32))
    router_w = np.ascontiguousarray(np.asarray(router_w, dtype=np.float32))
    weights1 = np.asarray(weights1, dtype=np.float32)
    weights2 = np.asarray(weights2, dtype=np.float32)
    assert tokens.shape == (T, D) and router_w.shape == (E, D)
    assert weights1.shape == (E, D, H) and weights2.shape == (E, H, O)

    comb = _route(tokens, router_w)  # [T, E] float64
    idx = [np.nonzero(comb[:, c])[0] for c in range(E)]
    maxL = max(len(i) for i in idx)
    C = max(64, -(-maxL // 32) * 32)

    if C not in _NC_CACHE:
        _NC_CACHE[C] = build(C)
    nc = _NC_CACHE[C]

    bf16 = ml_dtypes.bfloat16
    in_maps = []
    for c in range(E):
        # tokens pre-tiled to [p, a, t]: tokG[p, a, s] = tokens[idx[s], a*128+p]
        tokG = np.zeros((P, KD, C), dtype=bf16)
        g = tokens[idx[c]].T.reshape(KD, P, len(idx[c])).transpose(1, 0, 2)
        tokG[:, :, : len(idx[c])] = g.astype(bf16)
        # w1 pre-tiled to [j, p, (a hh)]: w1t[j, p, a*128+hh] = w1[a*128+p, j*128+hh]
        w1t = (
            weights1[c]
            .reshape(KD, P, JT, P)
            .transpose(2, 1, 0, 3)
            .reshape(JT, P, KD * P)
        )
        # w2 pre-tiled to [o, p, (j oo)], with the gelu 0.5 folded in
        w2t = (
            (weights2[c] * 0.5)
            .reshape(JT, P, OT, P)
            .transpose(2, 1, 0, 3)
            .reshape(OT, P, JT * P)
        )
        in_maps.append(
            {
                "tokG": tokG,
                "w1": np.ascontiguousarray(w1t).astype(bf16),
                "w2": np.ascontiguousarray(w2t).astype(bf16),
            }
        )

    res = run_bass_kernel_spmd(nc, in_maps, list(range(_NCORES)), trace=trace)
    out = np.zeros((T, O), dtype=np.float64)
    for c in range(E):
        yT = np.asarray(res.results[c]["out"]).astype(np.float64)  # [OT, P, C]
        L = len(idx[c])
        y = yT.reshape(O, C)[:, :L]
        out[idx[c]] += comb[idx[c], c : c + 1] * y.T
    if trace:
        kernel.last_results = res
    return out.astype(np.float32)


# revision 9
# speedup vs baseline: 1.1194x; 1.0093x over previous
"""MoE grouped-GEMM (router + top-2 combine + per-expert FFN) on 8 TRN2 NeuronCores.

Expert parallelism with token gather ("all-to-all tokens by expert assignment"):
the router (linear -> softmax -> top-2) runs host-side as part of the shard
step; core c owns expert c (weights1[c], weights2[c]) and receives ONLY the
tokens routed to expert c, padded to a common capacity C (max expert load
rounded up to 32). Each core computes its expert's FFN for its gathered
tokens; the host applies the combine weights and scatter-adds the 8 partial
outputs back to token order (the unshard step).

This cuts device FLOPs 4x vs the dense-over-experts formulation: only
top-2-of-8 expert-token pairs are computed (2048*2 = 4096 pairs vs 2048*8).

Problem shapes (hardcoded): tokens [2048, 1024] f32, router_w [8, 1024],
weights1 [8, 1024, 1024], weights2 [8, 1024, 1024], out [2048, 1024].

Per-core device program (SPMD, differs only via inputs):
  tokG [128, 8, C]     gathered tokens bf16, pre-tiled host-side so the
                       contraction dim d lands on SBUF partitions (p = d%128,
                       a = d//128) and every DMA run is >= 1KB contiguous.
  w1   [8, 128, 1024]  weights1[c] pre-tiled as [j, p, (a hh)] so a per-j
                       chunk DMA is one contiguous 2KB run per partition.
  w2   [8, 128, 1024]  weights2[c] * 0.5, pre-tiled as [o, p, (j oo)].
  FFN: hT[j, t] = x * (1 + erf(x/sqrt(2))),  x = sum_d w1[d, j] tokG[d, t]
       yT[o, t] = sum_j hT[j, t] w2[j, o]
  out  [8, 128, C]     yT bf16 (combine weights applied host-side).

Both GEMMs keep tokens as the moving operand (512-max free dim), so a
non-multiple-of-128 capacity wastes nothing on the PE. Input DMAs are
prefetch-ordered on the SP HWDGE queue (w1 j-chunk 0 + first tokens first);
w2 streams concurrently on the Activation HWDGE queue.
"""

import os
import sys

import numpy as np

for _p in ("/opt/trn_rl_repo", "/root/.axon_site/_ro/trn_rl_repo"):
    if os.path.isdir(_p) and _p not in sys.path:
        sys.path.insert(0, _p)

from contextlib import ExitStack

import concourse.bass as bass
import concourse.tile as tile
from concourse import bacc, mybir
from concourse.bass_utils import run_bass_kernel_spmd

F32 = mybir.dt.float32
BF16 = mybir.dt.bfloat16
AF = mybir.ActivationFunctionType
ALU = mybir.AluOpType

T = 2048  # tokens
D = 1024  # input dim
H = 1024  # hidden dim
O = 1024  # output dim
E = 8  # experts == cores
P = 128  # partitions
KD = D // P  # 8 contraction tiles (d)
JT = H // P  # 8 contraction tiles (j)
OT = O // P  # 8 output tiles (o)
_NCORES = 8


def _blocks(C):
    """Split C tokens into moving-dim blocks of <= 512 (PSUM bank limit),
    biggest first: a large block 0 makes GEMM1's j-cycle slower than the w1
    j-chunk DMA feed, so the PE never stalls on weight arrival."""
    out = []
    s = 0
    while s < C:
        tb = min(512, C - s)
        out.append((s, tb))
        s += tb
    return out


def _emit(tc, aps, act_fn, C):
    nc = tc.nc
    tokd = aps["tokG"]  # [P, KD, C]
    w1d = aps["w1"].rearrange("j p x -> p j x")  # [P, JT, KD*128]
    w2d = aps["w2"].rearrange("o p x -> p o x")  # [P, OT, JT*128]
    outd = aps["out"].rearrange("o p t -> p o t")  # [P, OT, C]
    blocks = _blocks(C)

    with ExitStack() as ctx:
        wp = ctx.enter_context(tc.tile_pool(name="wp", bufs=1))
        hp = ctx.enter_context(tc.tile_pool(name="hp", bufs=1))
        yp = ctx.enter_context(tc.tile_pool(name="yp", bufs=6))
        ph = ctx.enter_context(tc.tile_pool(name="ph", bufs=4, space="PSUM"))
        py = ctx.enter_context(tc.tile_pool(name="py", bufs=4, space="PSUM"))

        tok_sb = wp.tile([P, KD, C], BF16)
        w1_sb = wp.tile([P, JT, KD * P], BF16)
        w2_sb = wp.tile([P, OT, JT * P], BF16)

        # Input DMAs, split across the two HWDGE queues so both engine sets
        # pull concurrently, each in consumption order:
        #   SP queue:  w1 j-chunks 0..7 (GEMM1 stationaries), w2 o-chunks 4-7
        #   Act queue: block-0 tokens (a0-1 first so the first accumulation
        #              can start early), remaining tokens, w2 o-chunks 0-3,
        #              then the output stores emitted by the GEMM2 loop.
        for j in range(JT):
            nc.sync.dma_start(w1_sb[:, j, :], w1d[:, j, :])
        nc.sync.dma_start(w2_sb[:, 4:OT, :], w2d[:, 4:OT, :])
        b0, tb0 = blocks[0]
        nc.scalar.dma_start(tok_sb[:, 0:2, 0:tb0], tokd[:, 0:2, 0:tb0])
        nc.scalar.dma_start(tok_sb[:, 2:KD, 0:tb0], tokd[:, 2:KD, 0:tb0])
        for bs, tb in blocks[1:]:
            nc.scalar.dma_start(tok_sb[:, :, bs : bs + tb], tokd[:, :, bs : bs + tb])
        nc.scalar.dma_start(w2_sb[:, 0:4, :], w2d[:, 0:4, :])

        # PE warmup: the Tensor engine ramps its clock only after ~3us of
        # continuous execution. Dummy matmuls on a zeroed scratch tile keep
        # the PE busy during the initial DMA wait so the real GEMMs start at
        # (or near) full clock. One psum tile is reused (never read).
        wu_sb = wp.tile([P, 640], BF16)
        nc.vector.memset(wu_sb[:], 0.0)
        psum_wu = ph.tile([P, 512], F32, name="psum_h")
        for _ in range(10):
            nc.tensor.matmul(
                psum_wu[:], lhsT=wu_sb[:, 0:P], rhs=wu_sb[:, P : P + 512],
                start=True, stop=True, skip_group_check=True,
            )

        # ---- GEMM1: hT[j, t] = act(sum_d w1[d, j] tokG[d, t]) ----
        # Exact gelu(x) = 0.5*x*(1 + erf(x/sqrt(2))); the 0.5 is folded into
        # w2 host-side, so on-device: h = x * (1 + erf(x/sqrt(2))).
        h_sb = [
            hp.tile([P, JT, tb], BF16, name=f"h_sb{bi}")
            for bi, (_, tb) in enumerate(blocks)
        ]
        for bi, (bs, tb) in enumerate(blocks):
            for j in range(JT):
                psum_h = ph.tile([P, tb], F32, name="psum_h")
                for a in range(KD):
                    nc.tensor.matmul(
                        psum_h[:],
                        lhsT=w1_sb[:, j, a * P : (a + 1) * P],
                        rhs=tok_sb[:, a, bs : bs + tb],
                        start=(a == 0),
                        stop=(a == KD - 1),
                    )
                e_sb = yp.tile([P, tb], F32, name="e_sb")
                nc.scalar.activation(
                    e_sb[:], psum_h[:], act_fn, scale=0.7071067811865476
                )
                nc.vector.scalar_tensor_tensor(
                    h_sb[bi][:, j, :], e_sb[:], 1.0, psum_h[:],
                    op0=ALU.add, op1=ALU.mult,
                )

        # ---- GEMM2: yT[o, t] = sum_j hT[j, t] w2[j, o] ----
        for o in range(OT):
            y_sb = yp.tile([P, C], BF16, name="y_sb")
            for bi, (bs, tb) in enumerate(blocks):
                psum_y = py.tile([P, tb], F32, name="psum_y")
                for j in range(JT):
                    nc.tensor.matmul(
                        psum_y[:],
                        lhsT=w2_sb[:, o, j * P : (j + 1) * P],
                        rhs=h_sb[bi][:, j, :],
                        start=(j == 0),
                        stop=(j == JT - 1),
                    )
                nc.scalar.copy(y_sb[:, bs : bs + tb], psum_y[:])
            nc.scalar.dma_start(outd[:, o, :], y_sb[:])


def build(C, sim_act=False):
    """Build + compile the SPMD program for token capacity C. sim_act=True
    swaps the FFN activation to Tanh so CoreSim (which lacks Erf) can run."""
    nc = bacc.Bacc(
        "TRN2", target_bir_lowering=False, debug=False, num_devices=_NCORES
    )
    aps = {
        "tokG": nc.dram_tensor("tokG", [P, KD, C], BF16, kind="ExternalInput").ap(),
        "w1": nc.dram_tensor("w1", [JT, P, KD * P], BF16, kind="ExternalInput").ap(),
        "w2": nc.dram_tensor("w2", [OT, P, JT * P], BF16, kind="ExternalInput").ap(),
        "out": nc.dram_tensor("out", [OT, P, C], BF16, kind="ExternalOutput").ap(),
    }
    act = AF.Tanh if sim_act else AF.Erf
    with tile.TileContext(nc) as tc:
        _emit(tc, aps, act, C)
    nc.compile()
    return nc


def _route(tokens, router_w):
    """Host router in float64: linear -> softmax -> top-2. Margins on this
    input are ~1e-4, far above f32 eps, so selection matches the f32 ref."""
    logits = tokens.astype(np.float64) @ router_w.astype(np.float64).T  # [T, E]
    e = np.exp(logits - logits.max(axis=1, keepdims=True))
    scores = e / e.sum(axis=1, keepdims=True)
    order = np.argsort(scores, axis=1)
    ind = np.zeros_like(scores)
    np.put_along_axis(ind, order[:, -2:], 1.0, axis=1)
    return scores * ind  # comb [T, E]


_NC_CACHE = {}


def kernel(tokens, router_w, weights1, weights2, trace=False):
    import ml_dtypes

    tokens = np.ascontiguousarray(np.asarray(tokens, dtype=np.float32))
    router_w = np.ascontiguousarray(np.asarray(router_w, dtype=np.float32))
    weights1 = np.asarray(weights1, dtype=np.float32)
    weights2 = np.asarray(weights2, dtype=np.float32)
    assert tokens.shape == (T, D) and router_w.shape == (E, D)
    assert weights1.shape == (E, D, H) and weights2.shape == (E, H, O)

    comb = _route(tokens, router_w)  # [T, E] float64
    idx = [np.nonzero(comb[:, c])[0] for c in range(E)]
    maxL = max(len(i) for i in idx)
    C = max(64, -(-maxL // 32) * 32)

    if C not in _NC_CACHE:
        _NC_CACHE[C] = build(C)
    nc = _NC_CACHE[C]

    bf16 = ml_dtypes.bfloat16
    in_maps = []
    for c in range(E):
        # tokens pre-tiled to [p, a, t]: tokG[p, a, s] = tokens[idx[s], a*128+p]
        tokG = np.zeros((P, KD, C), dtype=bf16)
        g = tokens[idx[c]].T.reshape(KD, P, len(idx[c])).transpose(1, 0, 2)
        tokG[:, :, : len(idx[c])] = g.astype(bf16)
        # w1 pre-tiled to [j, p, (a hh)]: w1t[j, p, a*128+hh] = w1[a*128+p, j*128+hh]
        w1t = (
            weights1[c]
            .reshape(KD, P, JT, P)
            .transpose(2, 1, 0, 3)
            .reshape(JT, P, KD * P)
        )
        # w2 pre-tiled to [o, p, (j oo)], with the gelu 0.5 folded in
        w2t = (
            (weights2[c] * 0.5)
            .reshape(JT, P, OT, P)
            .transpose(2, 1, 0, 3)
            .reshape(OT, P, JT * P)
        )
        in_maps.append(
            {
                "tokG": tokG,
                "w1": np.ascontiguousarray(w1t).astype(bf16),
                "w2": np.ascontiguousarray(w2t).astype(bf16),
            }
        )

    res = run_bass_kernel_spmd(nc, in_maps, list(range(_NCORES)), trace=trace)
    out = np.zeros((T, O), dtype=np.float64)
    for c in range(E):
        yT = np.asarray(res.results[c]["out"]).astype(np.float64)  # [OT, P, C]
        L = len(idx[c])
        y = yT.reshape(O, C)[:, :L]
        out[idx[c]] += comb[idx[c], c : c + 1] * y.T
    if trace:
        kernel.last_results = res
    return out.astype(np.float32)


# revision 10
# speedup vs baseline: 1.1709x; 1.0460x over previous
"""MoE grouped-GEMM (router + top-2 combine + per-expert FFN) on 8 TRN2 NeuronCores.

Expert parallelism with token gather ("all-to-all tokens by expert assignment"):
the router (linear -> softmax -> top-2) runs host-side as part of the shard
step; core c owns expert c (weights1[c], weights2[c]) and receives ONLY the
tokens routed to expert c, padded to a common capacity C (max expert load
rounded up to 32). Each core computes its expert's FFN for its gathered
tokens; the host applies the combine weights and scatter-adds the 8 partial
outputs back to token order (the unshard step).

This cuts device FLOPs 4x vs the dense-over-experts formulation: only
top-2-of-8 expert-token pairs are computed (2048*2 = 4096 pairs vs 2048*8).

Problem shapes (hardcoded): tokens [2048, 1024] f32, router_w [8, 1024],
weights1 [8, 1024, 1024], weights2 [8, 1024, 1024], out [2048, 1024].

Per-core device program (SPMD, differs only via inputs):
  tokG [128, 8, C]     gathered tokens bf16, pre-tiled host-side so the
                       contraction dim d lands on SBUF partitions (p = d%128,
                       a = d//128) and every DMA run is >= 1KB contiguous.
  w1   [8, 128, 1024]  weights1[c] pre-tiled as [j, p, (a hh)] so a per-j
                       chunk DMA is one contiguous 2KB run per partition.
  w2   [8, 128, 1024]  weights2[c] * 0.5, pre-tiled as [o, p, (j oo)].
  FFN: hT[j, t] = x * (1 + erf(x/sqrt(2))),  x = sum_d w1[d, j] tokG[d, t]
       yT[o, t] = sum_j hT[j, t] w2[j, o]
  out  [8, 128, C]     yT bf16 (combine weights applied host-side).

Both GEMMs keep tokens as the moving operand (512-max free dim), so a
non-multiple-of-128 capacity wastes nothing on the PE. Input DMAs are
prefetch-ordered on the SP HWDGE queue (w1 j-chunk 0 + first tokens first);
w2 streams concurrently on the Activation HWDGE queue.
"""

import os
import sys

import numpy as np

for _p in ("/opt/trn_rl_repo", "/root/.axon_site/_ro/trn_rl_repo"):
    if os.path.isdir(_p) and _p not in sys.path:
        sys.path.insert(0, _p)

from contextlib import ExitStack

import concourse.bass as bass
import concourse.tile as tile
from concourse import bacc, mybir
from concourse.bass_utils import run_bass_kernel_spmd

F32 = mybir.dt.float32
BF16 = mybir.dt.bfloat16
AF = mybir.ActivationFunctionType
ALU = mybir.AluOpType

T = 2048  # tokens
D = 1024  # input dim
H = 1024  # hidden dim
O = 1024  # output dim
E = 8  # experts == cores
P = 128  # partitions
KD = D // P  # 8 contraction tiles (d)
JT = H // P  # 8 contraction tiles (j)
OT = O // P  # 8 output tiles (o)
_NCORES = 8


def _blocks(C):
    """Split C tokens into moving-dim blocks of <= 512 (PSUM bank limit),
    biggest first: a large block 0 makes GEMM1's j-cycle slower than the w1
    j-chunk DMA feed, so the PE never stalls on weight arrival."""
    out = []
    s = 0
    while s < C:
        tb = min(512, C - s)
        out.append((s, tb))
        s += tb
    return out


def _emit(tc, aps, act_fn, C):
    nc = tc.nc
    tokd = aps["tokG"]  # [P, KD, C]
    w1d = aps["w1"].rearrange("j p x -> p j x")  # [P, JT, KD*128]
    w2d = aps["w2"].rearrange("o p x -> p o x")  # [P, OT, JT*128]
    outd = aps["out"].rearrange("o p t -> p o t")  # [P, OT, C]
    blocks = _blocks(C)

    with ExitStack() as ctx:
        wp = ctx.enter_context(tc.tile_pool(name="wp", bufs=1))
        hp = ctx.enter_context(tc.tile_pool(name="hp", bufs=1))
        yp = ctx.enter_context(tc.tile_pool(name="yp", bufs=6))
        ph = ctx.enter_context(tc.tile_pool(name="ph", bufs=4, space="PSUM"))
        py = ctx.enter_context(tc.tile_pool(name="py", bufs=4, space="PSUM"))

        tok_sb = wp.tile([P, KD, C], BF16)
        w1_sb = wp.tile([P, JT, KD * P], BF16)
        w2_sb = wp.tile([P, OT, JT * P], BF16)

        # Input DMAs: all on the SP HWDGE queue, in exact PE consumption
        # order (the queue executes transfers in order; data starts flowing
        # only after the ~8us engine-init preamble, so the w1 j0 chunk plus
        # the first token chunk gate the first real matmul). w2 rides last:
        # GEMM2 starts only after all of GEMM1. Output stores go on the
        # Activation HWDGE queue so they never queue behind inputs.
        b0, tb0 = blocks[0]
        nc.sync.dma_start(w1_sb[:, 0, :], w1d[:, 0, :])
        nc.sync.dma_start(tok_sb[:, 0:2, 0:tb0], tokd[:, 0:2, 0:tb0])
        nc.sync.dma_start(tok_sb[:, 2:KD, 0:tb0], tokd[:, 2:KD, 0:tb0])
        for j in range(1, JT):
            nc.sync.dma_start(w1_sb[:, j, :], w1d[:, j, :])
        for bs, tb in blocks[1:]:
            nc.sync.dma_start(tok_sb[:, :, bs : bs + tb], tokd[:, :, bs : bs + tb])
        nc.sync.dma_start(w2_sb[:, 0:4, :], w2d[:, 0:4, :])
        nc.sync.dma_start(w2_sb[:, 4:OT, :], w2d[:, 4:OT, :])

        # PE warmup: the Tensor engine ramps its clock only after ~3us of
        # continuous execution. Dummy matmuls on a zeroed scratch tile keep
        # the PE busy during the initial DMA wait so the real GEMMs start at
        # (or near) full clock. One psum tile is reused (never read).
        wu_sb = wp.tile([P, 640], BF16)
        nc.vector.memset(wu_sb[:], 0.0)
        psum_wu = ph.tile([P, 512], F32, name="psum_h")
        for _ in range(8):
            nc.tensor.matmul(
                psum_wu[:], lhsT=wu_sb[:, 0:P], rhs=wu_sb[:, P : P + 512],
                start=True, stop=True, skip_group_check=True,
            )

        # ---- GEMM1: hT[j, t] = act(sum_d w1[d, j] tokG[d, t]) ----
        # Exact gelu(x) = 0.5*x*(1 + erf(x/sqrt(2))); the 0.5 is folded into
        # w2 host-side, so on-device: h = x * (1 + erf(x/sqrt(2))).
        h_sb = [
            hp.tile([P, JT, tb], BF16, name=f"h_sb{bi}")
            for bi, (_, tb) in enumerate(blocks)
        ]
        for bi, (bs, tb) in enumerate(blocks):
            for j in range(JT):
                psum_h = ph.tile([P, tb], F32, name="psum_h")
                for a in range(KD):
                    nc.tensor.matmul(
                        psum_h[:],
                        lhsT=w1_sb[:, j, a * P : (a + 1) * P],
                        rhs=tok_sb[:, a, bs : bs + tb],
                        start=(a == 0),
                        stop=(a == KD - 1),
                    )
                e_sb = yp.tile([P, tb], F32, name="e_sb")
                nc.scalar.activation(
                    e_sb[:], psum_h[:], act_fn, scale=0.7071067811865476
                )
                nc.vector.scalar_tensor_tensor(
                    h_sb[bi][:, j, :], e_sb[:], 1.0, psum_h[:],
                    op0=ALU.add, op1=ALU.mult,
                )

        # ---- GEMM2: yT[o, t] = sum_j hT[j, t] w2[j, o] ----
        for o in range(OT):
            y_sb = yp.tile([P, C], BF16, name="y_sb")
            for bi, (bs, tb) in enumerate(blocks):
                psum_y = py.tile([P, tb], F32, name="psum_y")
                for j in range(JT):
                    nc.tensor.matmul(
                        psum_y[:],
                        lhsT=w2_sb[:, o, j * P : (j + 1) * P],
                        rhs=h_sb[bi][:, j, :],
                        start=(j == 0),
                        stop=(j == JT - 1),
                    )
                nc.scalar.copy(y_sb[:, bs : bs + tb], psum_y[:])
            nc.scalar.dma_start(outd[:, o, :], y_sb[:])


def build(C, sim_act=False):
    """Build + compile the SPMD program for token capacity C. sim_act=True
    swaps the FFN activation to Tanh so CoreSim (which lacks Erf) can run."""
    nc = bacc.Bacc(
        "TRN2", target_bir_lowering=False, debug=False, num_devices=_NCORES
    )
    aps = {
        "tokG": nc.dram_tensor("tokG", [P, KD, C], BF16, kind="ExternalInput").ap(),
        "w1": nc.dram_tensor("w1", [JT, P, KD * P], BF16, kind="ExternalInput").ap(),
        "w2": nc.dram_tensor("w2", [OT, P, JT * P], BF16, kind="ExternalInput").ap(),
        "out": nc.dram_tensor("out", [OT, P, C], BF16, kind="ExternalOutput").ap(),
    }
    act = AF.Tanh if sim_act else AF.Erf
    with tile.TileContext(nc) as tc:
        _emit(tc, aps, act, C)
    nc.compile()
    return nc


def _route(tokens, router_w):
    """Host router in float64: linear -> softmax -> top-2. Margins on this
    input are ~1e-4, far above f32 eps, so selection matches the f32 ref."""
    logits = tokens.astype(np.float64) @ router_w.astype(np.float64).T  # [T, E]
    e = np.exp(logits - logits.max(axis=1, keepdims=True))
    scores = e / e.sum(axis=1, keepdims=True)
    order = np.argsort(scores, axis=1)
    ind = np.zeros_like(scores)
    np.put_along_axis(ind, order[:, -2:], 1.0, axis=1)
    return scores * ind  # comb [T, E]


_NC_CACHE = {}


def kernel(tokens, router_w, weights1, weights2, trace=False):
    import ml_dtypes

    tokens = np.ascontiguousarray(np.asarray(tokens, dtype=np.float32))
    router_w = np.ascontiguousarray(np.asarray(router_w, dtype=np.float32))
    weights1 = np.asarray(weights1, dtype=np.float32)
    weights2 = np.asarray(weights2, dtype=np.float32)
    assert tokens.shape == (T, D) and router_w.shape == (E, D)
    assert weights1.shape == (E, D, H) and weights2.shape == (E, H, O)

    comb = _route(tokens, router_w)  # [T, E] float64
    idx = [np.nonzero(comb[:, c])[0] for c in range(E)]
    maxL = max(len(i) for i in idx)
    C = max(64, -(-maxL // 32) * 32)

    if C not in _NC_CACHE:
        _NC_CACHE[C] = build(C)
    nc = _NC_CACHE[C]

    bf16 = ml_dtypes.bfloat16
    in_maps = []
    for c in range(E):
        # tokens pre-tiled to [p, a, t]: tokG[p, a, s] = tokens[idx[s], a*128+p]
        tokG = np.zeros((P, KD, C), dtype=bf16)
        g = tokens[idx[c]].T.reshape(KD, P, len(idx[c])).transpose(1, 0, 2)
        tokG[:, :, : len(idx[c])] = g.astype(bf16)
        # w1 pre-tiled to [j, p, (a hh)]: w1t[j, p, a*128+hh] = w1[a*128+p, j*128+hh]
        w1t = (
            weights1[c]
            .reshape(KD, P, JT, P)
            .transpose(2, 1, 0, 3)
            .reshape(JT, P, KD * P)
        )
        # w2 pre-tiled to [o, p, (j oo)], with the gelu 0.5 folded in
        w2t = (
            (weights2[c] * 0.5)
            .reshape(JT, P, OT, P)
            .transpose(2, 1, 0, 3)
            .reshape(OT, P, JT * P)
        )
        in_maps.append(
            {
                "tokG": tokG,
                "w1": np.ascontiguousarray(w1t).astype(bf16),
                "w2": np.ascontiguousarray(w2t).astype(bf16),
            }
        )

    res = run_bass_kernel_spmd(nc, in_maps, list(range(_NCORES)), trace=trace)
    out = np.zeros((T, O), dtype=np.float64)
    for c in range(E):
        yT = np.asarray(res.results[c]["out"]).astype(np.float64)  # [OT, P, C]
        L = len(idx[c])
        y = yT.reshape(O, C)[:, :L]
        out[idx[c]] += comb[idx[c], c : c + 1] * y.T
    if trace:
        kernel.last_results = res
    return out.astype(np.float32)
